# Initial kernel scaffold
#
"""Trainium2 Bass kernel for nn_DAFCN (motion-prediction DAFCN forward).

Structure exploited (verified vs the reference):
  * The attention branch (wq*/wk* convs, dvb) is dead code: the reference
    computes `combined[:, :, :DCT_N]` which selects only the GCN output.
  * The FFC branch (rfft -> 1x1 conv -> relu -> irfft, first 10 steps) is
    linear-relu-linear and is folded into two small matmuls (M1f, M2/M3).
  * The iDCT + MLP are folded: h = relu(gcn_out @ A1 + ffc10 @ B1 + hb),
    out = h @ W2  with  A1 = (mlp_w1[:, :30] @ idct[:, :10]).T,
    B1 = mlp_w1[:, 30:40].T, W2 = mlp_w2[:10].T, and gc7_w folded into
    W7A = gc7_w @ A1, hb = gc7_b @ A1.

Sharding: pure data parallelism — 1024 samples / 8 cores = 128 per core,
weights replicated.

Device dataflow per core (all matmuls on PE, f32r):
  * GCN state y kept transposed ("layout B": feature chunks on partitions,
    (sample, node) on the free axis), 2 samples (96 rows) per matmul group.
  * feature mix:  u = y @ w     via lhsT = y^T chunk, rhs = w chunk  (PSUM acc)
  * node mix:     z^T = u^T att^T via lhsT = u rows,  rhs = att^T    -> layout B
  * tanh+bias on ScalarE (bias per partition), residuals on VectorE.
"""

import numpy as np

import concourse.bass as bass
from concourse import mybir
from concourse.tile import TileContext

F32 = mybir.dt.float32
F32R = mybir.dt.float32r
AF = mybir.ActivationFunctionType

N_CORES = 8
B_TOT, T_IN, F_FEAT = 1024, 50, 48
SPC = B_TOT // N_CORES          # samples per core
D = 512
DCT_N = 10


# --------------------------------------------------------------------------
# host-side constant folding
# --------------------------------------------------------------------------

def _build_host_consts(inp):
    f8 = np.float64
    w1 = np.asarray(inp["mlp_w1"], f8)       # [256, 40]
    w2 = np.asarray(inp["mlp_w2"], f8)       # [40, 256]
    wg = np.asarray(inp["ffc_wg"], f8)       # [6, 6]
    wl = np.asarray(inp["ffc_wl"], f8)       # [3, 3]
    gc1_w = np.asarray(inp["gc1_w"], f8)     # [10, 512]
    gc1_b = np.asarray(inp["gc1_b"], f8)     # [512]
    gcb_w = np.asarray(inp["gcb_w"], f8)     # [2, 2, 512, 512]
    gcb_b = np.asarray(inp["gcb_b"], f8)     # [2, 2, 512]
    gc7_w = np.asarray(inp["gc7_w"], f8)     # [512, 10]
    gc7_b = np.asarray(inp["gc7_b"], f8)     # [10]
    att1 = np.asarray(inp["gc1_att"], f8)    # [48, 48]
    attb = np.asarray(inp["gcb_att"], f8)    # [2, 2, 48, 48]
    att7 = np.asarray(inp["gc7_att"], f8)    # [48, 48]

    # DCT pair (block length 30)
    N = 30
    kk = np.arange(N)[:, None]
    ii = np.arange(N)[None, :]
    w = np.full((N, 1), np.sqrt(2.0 / N))
    w[0, 0] = np.sqrt(1.0 / N)
    d = w * np.cos(np.pi * (ii + 0.5) * kk / N)
    idct = np.linalg.inv(d)
    dct10 = d[:DCT_N]                        # [10, 30]

    # E: x^T[d, f] = sum_j E[d, j] * seq[40+j, f]
    E = dct10[:, :10].copy()
    E[:, 9] += dct10[:, 10:].sum(axis=1)

    # MLP folds
    A1 = (w1[:, :30] @ idct[:, :10]).T       # [10, 256]
    B1 = w1[:, 30:40].T                      # [10, 256]
    W7A = gc7_w @ A1                         # [512, 256]
    hb = gc7_b @ A1                          # [256]
    W2 = w2[:10].T                           # [256, 10]

    # FFC fold: rfft / channel mix / (relu) / irfft+local, first 10 steps
    Fm = np.fft.rfft(np.eye(60), axis=-1)    # [60, 31]
    Fr, Fi = Fm.real.T, Fm.imag.T            # [31, 60]
    M1 = (np.einsum("oc,kt->ctok", wg[:, :3], Fr)
          + np.einsum("oc,kt->ctok", wg[:, 3:], Fi)).reshape(3, 60, 186)
    M1f = np.concatenate(
        [M1[:, :49], M1[:, 49:].sum(axis=1, keepdims=True)], axis=1
    ).reshape(150, 186)                      # [(c,t<50), (o,k)]
    Gr = np.fft.irfft(np.eye(31), n=60, axis=-1)[:, :10]
    Gi = np.fft.irfft(1j * np.eye(31), n=60, axis=-1)[:, :10]
    M2 = np.zeros((6, 31, 3, 10))
    for o3 in range(3):
        M2[o3, :, o3, :] = Gr
        M2[o3 + 3, :, o3, :] = Gi
    M2 = M2.reshape(186, 30)
    M3 = np.einsum("oc,tu->ctou", wl, np.eye(10)).reshape(30, 30)

    import ml_dtypes
    c = {}
    f4 = lambda a: np.ascontiguousarray(a, ml_dtypes.bfloat16)

    # WGCB [128, 16, 512]: (layer l, k-chunk kc) -> w_l[kc*128+p, j]
    WGCB = np.zeros((128, 16, 512))
    for layer in range(4):
        s, ll = divmod(layer, 2)
        wl_ = gcb_w[s, ll]
        for kc in range(4):
            WGCB[:, layer * 4 + kc, :] = wl_[kc * 128:(kc + 1) * 128]
    c["WGCB"] = f4(WGCB)

    W7At = np.zeros((128, 4, 256))
    for kc in range(4):
        W7At[:, kc, :] = W7A[kc * 128:(kc + 1) * 128]
    c["W7AT"] = f4(W7At)

    c["WG1"] = f4(gc1_w)                     # [10, 512]
    A1B1 = np.zeros((10, 512))
    A1B1[:, :256] = A1
    A1B1[:, 256:] = B1
    c["A1B1"] = f4(A1B1)

    W2T = np.zeros((128, 2, 10))
    for mc in range(2):
        W2T[:, mc, :] = W2[mc * 128:(mc + 1) * 128]
    c["W2T"] = f4(W2T)

    c["ET"] = f4(E.T)                        # [10, 10] lhsT for dct matmul

    M1S = np.zeros((50, 3, 186))
    for cc in range(3):
        M1S[:, cc, :] = M1f[cc * 50:(cc + 1) * 50]
    c["M1S"] = f4(M1S)

    M2S = np.zeros((128, 2, 3, 10))
    M2S[:, 0] = M2[:128].reshape(128, 3, 10)
    M2S[:58, 1] = M2[128:].reshape(58, 3, 10)
    c["M2S"] = f4(M2S)

    # M3S[t, c, o3, t'] = M3[(c,t), (o3,t')]
    M3S = M3.reshape(3, 10, 3, 10).transpose(1, 0, 2, 3)
    c["M3S"] = f4(M3S)

    # block-diagonal (2-sample) transposed attention, bf16
    ATT2 = np.zeros((96, 6, 96))
    atts = [att1, attb[0, 0], attb[0, 1], attb[1, 0], attb[1, 1], att7]
    for i, a in enumerate(atts):
        ATT2[0:48, i, 0:48] = a.T
        ATT2[48:96, i, 48:96] = a.T
    c["ATT2"] = np.ascontiguousarray(ATT2, ml_dtypes.bfloat16)

    BIAS = np.zeros((128, 22))
    tanh_biases = [gc1_b, gcb_b[0, 0], gcb_b[0, 1], gcb_b[1, 0], gcb_b[1, 1]]
    for li, b in enumerate(tanh_biases):
        for mc in range(4):
            BIAS[:, li * 4 + mc] = b[mc * 128:(mc + 1) * 128]
    for mc in range(2):
        BIAS[:, 20 + mc] = hb[mc * 128:(mc + 1) * 128]
    c["BIAS"] = np.ascontiguousarray(BIAS, np.float32)
    return c


BF16 = mybir.dt.bfloat16

CONST_SPECS = {
    "WGCB": ((128, 16, 512), BF16),
    "W7AT": ((128, 4, 256), BF16),
    "WG1": ((10, 512), BF16),
    "A1B1": ((10, 512), BF16),
    "W2T": ((128, 2, 10), BF16),
    "ET": ((10, 10), BF16),
    "M1S": ((50, 3, 186), BF16),
    "M2S": ((128, 2, 3, 10), BF16),
    "M3S": ((10, 3, 3, 10), BF16),
    "ATT2": ((96, 6, 96), BF16),
    "BIAS": ((128, 22), F32),
}


# --------------------------------------------------------------------------
# bass program
# --------------------------------------------------------------------------

def _split_matmul_waits(raw):
    """TRN2 walrus codegen allows only one sync-wait on Matmult/Ldweights.

    Move extra waits onto EventSemaphore instructions inserted just before
    (same engine, in-order execution => semantics preserved).
    """
    import json as _json
    bir = _json.loads(raw)
    for fn in bir["functions"]:
        for bb in fn["blocks"]:
            out = []
            for inst in bb["instructions"]:
                si = inst.get("sync_info")
                if (inst.get("opcode") != "EventSemaphore"
                        and si and len(si.get("on_wait") or []) > 1):
                    waits = si["on_wait"]
                    keep, extras = waits[-1], waits[:-1]
                    ip = len(out)
                    if (inst["opcode"] == "Matmult" and out
                            and out[-1].get("opcode") == "Ldweights"
                            and out[-1].get("engine") == inst["engine"]
                            and not (out[-1].get("sync_info") or {}).get(
                                "on_wait")):
                        ip = len(out) - 1
                    for j, w in enumerate(extras):
                        out.insert(ip + j, {
                            "debug": inst.get("debug", 0),
                            "engine": inst["engine"],
                            "ins": [], "outs": [],
                            "name": f"{inst['name']}_ws{j}",
                            "opcode": "EventSemaphore",
                            "sync_info": {"on_update": [], "on_wait": [w]},
                        })
                    si["on_wait"] = [keep]
                out.append(inst)
            bb["instructions"] = out
    return _json.dumps(bir).encode()


def build_nc(spc=SPC):
    """Build the per-core Bass program for `spc` samples (multiple of 16)."""
    assert spc % 16 == 0
    n_sg = spc // 16
    nc = bass.Bass()

    xh = nc.declare_dram_parameter("xseq", [spc, T_IN, F_FEAT], BF16, isOutput=False)
    ch = {
        name: nc.declare_dram_parameter(name, list(shape), dt_, isOutput=False)
        for name, (shape, dt_) in CONST_SPECS.items()
    }
    oh = nc.declare_dram_parameter("out", [spc, DCT_N, 1, F_FEAT], F32, isOutput=True)

    with TileContext(nc) as tc:
        with (
            tc.tile_pool(name="consts", bufs=1) as consts,
            tc.tile_pool(name="seq", bufs=2) as p_seq,
            tc.tile_pool(name="zsb", bufs=2) as p_zsb,
            tc.tile_pool(name="ffc", bufs=2) as p_ffc,
            tc.tile_pool(name="dct", bufs=4) as p_dct,
            tc.tile_pool(name="ysb", bufs=4) as p_y,
            tc.tile_pool(name="hsb", bufs=3) as p_h,
            tc.tile_pool(name="psb", bufs=8) as p_p,
            tc.tile_pool(name="hbig", bufs=2) as p_hbig,
            tc.tile_pool(name="osb", bufs=2) as p_osb,
            tc.tile_pool(name="ps_u", bufs=2, space="PSUM") as ps_u,
            tc.tile_pool(name="ps_zt", bufs=4, space="PSUM") as ps_zt,
            tc.tile_pool(name="ps_big", bufs=2, space="PSUM") as ps_big,
        ):
            # ---- load constants ----
            W = {}
            for name, (shape, dt_) in CONST_SPECS.items():
                t = consts.tile(list(shape), dt_, tag=name)
                nc.sync.dma_start(out=t[...], in_=ch[name][...])
                W[name] = t

            def mm(out, lhsT, rhs, start=True, stop=True):
                nc.tensor.matmul(out=out, lhsT=lhsT, rhs=rhs,
                                 start=start, stop=stop)

            mmb = mm

            for sg in range(n_sg):
                g0 = sg * 16
                # ---- load 16 samples: seq [50, 16, 48] (t on partitions) ----
                seq = p_seq.tile([T_IN, 16, F_FEAT], BF16, tag="seq")
                nc.sync.dma_start(
                    out=seq[...],
                    in_=xh[g0:g0 + 16].rearrange("b t f -> t b f"),
                )
                # rows 40:50 again at base partition 0 (matmul operands
                # must start at partition 0/32/64)
                seq40 = p_seq.tile([10, 16, F_FEAT], BF16, tag="seq40")
                nc.sync.dma_start(
                    out=seq40[...],
                    in_=xh[g0:g0 + 16, 40:50].rearrange("b t f -> t b f"),
                )

                # ---- FFC: Z = relu(M1f^T @ Xg^T) ----
                zp = ps_big.tile([128, 2, 256], F32, tag="big", name="zp")
                for mc, msz in ((0, 128), (1, 58)):
                    for cc in range(3):
                        mm(zp[0:msz, mc, :],
                           W["M1S"][:, cc, mc * 128:mc * 128 + msz],
                           seq[:, :, cc * 16:(cc + 1) * 16],
                           start=(cc == 0), stop=(cc == 2))
                zsb = p_zsb.tile([128, 2, 256], BF16, tag="zsb")
                nc.scalar.activation(zsb[:, 0, :], zp[:, 0, :], AF.Relu)
                nc.scalar.activation(zsb[0:58, 1, :], zp[0:58, 1, :], AF.Relu)

                # ---- FFC second stage, per o3 -> ffc_all [10=t', (s,o3,g)] ----
                ffc_all = p_ffc.tile([10, 16, 3, 16], BF16, tag="ffc")
                for o3 in range(3):
                    fp = ps_big.tile([10, 256], F32, tag="big", name="fp")
                    mm(fp[...], W["M2S"][:, 0, o3, :], zsb[:, 0, :],
                       start=True, stop=False)
                    mm(fp[...], W["M2S"][0:58, 1, o3, :], zsb[0:58, 1, :],
                       start=False, stop=False)
                    for cc in range(3):
                        mm(fp[...], W["M3S"][:, cc, o3, :],
                           seq[0:10, :, cc * 16:(cc + 1) * 16],
                           start=False, stop=(cc == 2))
                    nc.vector.tensor_copy(
                        ffc_all[:, :, o3, :],
                        fp[...].rearrange("p (s g) -> p s g", g=16))

                def gcn_layer(dct, g8, src_y, layer, out_pool, out_tag):
                    """One GCN layer for one 8-sample group.

                    src_y: None (gc1: input is dct) or [128, 4, 384] tile.
                    Returns tanh output [128, 4(kc), 384=(pair,si,node)].
                    """
                    zts = [ps_zt.tile([128, 4, 96], F32, tag="zt",
                                      name=f"zt{mc}")
                           for mc in range(4)]
                    for pair in range(4):
                        u = ps_u.tile([96, 512], F32, tag="u", name="u")
                        if src_y is None:
                            mm(u[...], dct[:, pair * 96:(pair + 1) * 96],
                               W["WG1"][...])
                        else:
                            for kc in range(4):
                                mm(u[...],
                                   src_y[:, kc, pair * 96:(pair + 1) * 96],
                                   W["WGCB"][:, (layer - 1) * 4 + kc, :],
                                   start=(kc == 0), stop=(kc == 3))
                        p = p_p.tile([96, 512], BF16, tag="p", name="p")
                        if pair == 0:
                            nc.scalar.copy(p[...], u[...])
                        else:
                            nc.vector.tensor_copy(p[...], u[...])
                        for mc in range(4):
                            mmb(zts[mc][:, pair, :],
                                p[:, mc * 128:(mc + 1) * 128],
                                W["ATT2"][:, layer, :],
                                start=True, stop=True)
                    out8 = out_pool.tile([128, 4, 384], BF16, tag=out_tag,
                                         name=out_tag)
                    for mc in range(4):
                        col = layer * 4 + mc
                        nc.scalar.activation(
                            out8[:, mc, :], zts[mc][...], AF.Tanh,
                            bias=W["BIAS"][:, col:col + 1])
                    return out8

                # ---- dct per 8-group: x^T = E @ seq[40:50] -> [10, 384] ----
                dcts = []
                for g8 in range(2):
                    dp = ps_big.tile([10, 384], F32, tag="big", name="dp")
                    mm(dp[...], W["ET"][...], seq40[:, g8 * 8:g8 * 8 + 8, :])
                    dct = p_dct.tile([10, 384], BF16, tag="dct")
                    nc.vector.tensor_copy(dct[...], dp[...])
                    dcts.append(dct)

                # ---- GCN layers, two 8-groups in lockstep ----
                y8s = [gcn_layer(dcts[g8], g8, None, 0, p_y, "y8")
                       for g8 in range(2)]
                for st in range(2):
                    has = [gcn_layer(dcts[g8], g8, y8s[g8], 1 + st * 2,
                                     p_h, "h8") for g8 in range(2)]
                    hbs = [gcn_layer(dcts[g8], g8, has[g8], 2 + st * 2,
                                     p_h, "h8") for g8 in range(2)]
                    for g8 in range(2):
                        ynew = p_y.tile([128, 4, 384], BF16, tag="y8",
                                        name="ynew")
                        nc.gpsimd.tensor_add(ynew[...], y8s[g8][...],
                                             hbs[g8][...])
                        y8s[g8] = ynew

                # ---- gc7 + MLP per 8-group ----
                for g8 in range(2):
                    s0 = g8 * 8
                    dct, y8 = dcts[g8], y8s[g8]
                    hp = [ps_zt.tile([128, 8, 3, 16], F32, tag="zt",
                                     name=f"hp{mc}")
                          for mc in range(2)]
                    # term2 = (x @ A1)^T  (covers all elements; start=True)
                    for mc in range(2):
                        mm(hp[mc][...],
                           W["A1B1"][:, mc * 128:(mc + 1) * 128],
                           dct[...], start=True, stop=False)
                    # z7 = (att7 @ (y @ W7A))^T
                    for pair in range(4):
                        u7 = ps_u.tile([96, 256], F32, tag="u", name="u7")
                        for kc in range(4):
                            mm(u7[...], y8[:, kc, pair * 96:(pair + 1) * 96],
                               W["W7AT"][:, kc, :],
                               start=(kc == 0), stop=(kc == 3))
                        p7 = p_p.tile([96, 512], BF16, tag="p", name="p7")
                        if pair == 0:
                            nc.scalar.copy(p7[:, 0:256], u7[...])
                        else:
                            nc.vector.tensor_copy(p7[:, 0:256], u7[...])
                        for mc in range(2):
                            mmb(hp[mc][:, pair * 2:pair * 2 + 2, :, :],
                                p7[:, mc * 128:(mc + 1) * 128],
                                W["ATT2"][:, 5, :], start=False, stop=False)
                    # term3 = (ffc10 @ B1)^T  (rhs cols already (s, o3, g))
                    for mc in range(2):
                        mm(hp[mc][...],
                           W["A1B1"][:, 256 + mc * 128:256 + (mc + 1) * 128],
                           ffc_all[:, s0:s0 + 8, :, :],
                           start=False, stop=True)
                    # relu(h + hb) on DVE
                    hsb = p_hbig.tile([128, 2, 384], BF16, tag="hbig")
                    for mc in range(2):
                        nc.vector.tensor_scalar(
                            out=hsb[:, mc, :],
                            in0=hp[mc][...],
                            scalar1=W["BIAS"][:, 20 + mc:21 + mc],
                            scalar2=0.0,
                            op0=mybir.AluOpType.add,
                            op1=mybir.AluOpType.max)
                    # out = (h @ W2)^T -> [10, (s,f)]
                    op = ps_big.tile([10, 384], F32, tag="big", name="op")
                    for mc in range(2):
                        mm(op[...], W["W2T"][:, mc, :], hsb[:, mc, :],
                           start=(mc == 0), stop=(mc == 1))
                    osb = p_osb.tile([10, 384], F32, tag="osb")
                    nc.vector.tensor_copy(osb[...], op[...])
                    nc.sync.dma_start(
                        out=oh[g0 + s0:g0 + s0 + 8].rearrange(
                            "b t o f -> t b (o f)"),
                        in_=osb.rearrange("p (s f) -> p s f", f=F_FEAT),
                    )
    _orig_to_json_bytes = nc.to_json_bytes
    nc.to_json_bytes = lambda: _split_matmul_waits(_orig_to_json_bytes())
    return nc


# --------------------------------------------------------------------------
# host entry point
# --------------------------------------------------------------------------

_CACHE = {}


def kernel(**inputs):
    assert int(inputs.get("input_n", 50)) == 50
    assert int(inputs.get("output_n", 20)) == 20
    assert int(inputs.get("itera", 1)) == 1

    import ml_dtypes
    x = np.ascontiguousarray(
        np.asarray(inputs["input_seq"], np.float32).astype(ml_dtypes.bfloat16))
    assert x.shape == (B_TOT, T_IN, F_FEAT)

    consts = _build_host_consts(inputs)

    if "nc" not in _CACHE:
        _CACHE["nc"] = build_nc(SPC)
    nc = _CACHE["nc"]

    from concourse.bass_utils import run_bass_kernel_spmd

    in_maps = []
    for i in range(N_CORES):
        m = dict(consts)
        m["xseq"] = x[i * SPC:(i + 1) * SPC]
        in_maps.append(m)

    res = run_bass_kernel_spmd(nc, in_maps, list(range(N_CORES)))
    out = np.concatenate([res.results[i]["out"] for i in range(N_CORES)], axis=0)
    return out.astype(np.float32)



# revision 7
# speedup vs baseline: 1.5717x; 1.5717x over previous
"""Trainium2 Bass kernel for nn_DAFCN (motion-prediction DAFCN forward).

Structure exploited (verified vs the reference):
  * The attention branch (wq*/wk* convs, dvb) is dead code: the reference
    computes `combined[:, :, :DCT_N]` which selects only the GCN output.
  * The FFC branch (rfft -> 1x1 conv -> relu -> irfft, first 10 steps) is
    linear-relu-linear and is folded into two small matmuls (M1f, M2/M3).
  * The iDCT + MLP are folded: h = relu(gcn_out @ A1 + ffc10 @ B1 + hb),
    out = h @ W2  with  A1 = (mlp_w1[:, :30] @ idct[:, :10]).T,
    B1 = mlp_w1[:, 30:40].T, W2 = mlp_w2[:10].T, and gc7_w folded into
    W7A = gc7_w @ A1, hb = gc7_b @ A1.

Sharding: pure data parallelism — 1024 samples / 8 cores = 128 per core,
weights replicated.

Device dataflow per core:
  * GCN state y kept transposed ("layout B": feature chunks on partitions,
    (sample, node) on the free axis), 2 samples (96 rows) per matmul group.
  * feature mix:  u = y @ w   as fp8e4 DoubleRow matmuls (K=256 per mm)
  * node mix:     z^T = u^T att^T via lhsT = u rows, rhs = att^T (bf16)
  * tanh+bias on ScalarE writing fp8 state; residuals on gpsimd/DVE.
  * Emission is wave-ordered (f0 f1 a0 f2 a1 f3 a2 a3) so the PE never
    waits long on the PSUM->SBUF copies feeding the node-mix matmuls;
    copies are spread across Scalar/DVE/Pool.
  * All input DMAs are issued up front so loads never queue behind
    output stores on the SP DMA queue.
"""

import numpy as np

import concourse.bass as bass
from concourse import mybir
from concourse.tile import TileContext

F32 = mybir.dt.float32
BF16 = mybir.dt.bfloat16
FP8 = mybir.dt.float8e4
AF = mybir.ActivationFunctionType
DR = mybir.MatmulPerfMode.DoubleRow

N_CORES = 8
B_TOT, T_IN, F_FEAT = 1024, 50, 48
SPC = B_TOT // N_CORES          # samples per core
D = 512
DCT_N = 10


# --------------------------------------------------------------------------
# host-side constant folding
# --------------------------------------------------------------------------

def _build_host_consts(inp):
    f8 = np.float64
    w1 = np.asarray(inp["mlp_w1"], f8)       # [256, 40]
    w2 = np.asarray(inp["mlp_w2"], f8)       # [40, 256]
    wg = np.asarray(inp["ffc_wg"], f8)       # [6, 6]
    wl = np.asarray(inp["ffc_wl"], f8)       # [3, 3]
    gc1_w = np.asarray(inp["gc1_w"], f8)     # [10, 512]
    gc1_b = np.asarray(inp["gc1_b"], f8)     # [512]
    gcb_w = np.asarray(inp["gcb_w"], f8)     # [2, 2, 512, 512]
    gcb_b = np.asarray(inp["gcb_b"], f8)     # [2, 2, 512]
    gc7_w = np.asarray(inp["gc7_w"], f8)     # [512, 10]
    gc7_b = np.asarray(inp["gc7_b"], f8)     # [10]
    att1 = np.asarray(inp["gc1_att"], f8)    # [48, 48]
    attb = np.asarray(inp["gcb_att"], f8)    # [2, 2, 48, 48]
    att7 = np.asarray(inp["gc7_att"], f8)    # [48, 48]

    # DCT pair (block length 30)
    N = 30
    kk = np.arange(N)[:, None]
    ii = np.arange(N)[None, :]
    w = np.full((N, 1), np.sqrt(2.0 / N))
    w[0, 0] = np.sqrt(1.0 / N)
    d = w * np.cos(np.pi * (ii + 0.5) * kk / N)
    idct = np.linalg.inv(d)
    dct10 = d[:DCT_N]                        # [10, 30]

    # E: x^T[d, f] = sum_j E[d, j] * seq[40+j, f]
    E = dct10[:, :10].copy()
    E[:, 9] += dct10[:, 10:].sum(axis=1)

    # MLP folds
    A1 = (w1[:, :30] @ idct[:, :10]).T       # [10, 256]
    B1 = w1[:, 30:40].T                      # [10, 256]
    W7A = gc7_w @ A1                         # [512, 256]
    hb = gc7_b @ A1                          # [256]
    W2 = w2[:10].T                           # [256, 10]

    # FFC fold: rfft / channel mix / (relu) / irfft+local, first 10 steps
    Fm = np.fft.rfft(np.eye(60), axis=-1)    # [60, 31]
    Fr, Fi = Fm.real.T, Fm.imag.T            # [31, 60]
    M1 = (np.einsum("oc,kt->ctok", wg[:, :3], Fr)
          + np.einsum("oc,kt->ctok", wg[:, 3:], Fi)).reshape(3, 60, 186)
    M1f = np.concatenate(
        [M1[:, :49], M1[:, 49:].sum(axis=1, keepdims=True)], axis=1
    ).reshape(150, 186)                      # [(c,t<50), (o,k)]
    Gr = np.fft.irfft(np.eye(31), n=60, axis=-1)[:, :10]
    Gi = np.fft.irfft(1j * np.eye(31), n=60, axis=-1)[:, :10]
    M2 = np.zeros((6, 31, 3, 10))
    for o3 in range(3):
        M2[o3, :, o3, :] = Gr
        M2[o3 + 3, :, o3, :] = Gi
    M2 = M2.reshape(186, 30)
    M3 = np.einsum("oc,tu->ctou", wl, np.eye(10)).reshape(30, 30)

    import ml_dtypes
    c = {}
    f4 = lambda a: np.ascontiguousarray(a, ml_dtypes.bfloat16)
    q8 = lambda a: np.ascontiguousarray(a, ml_dtypes.float8_e4m3)

    # WGCB [128, 16, 512]: (layer l, k-chunk kc) -> w_l[kc*128+p, j], fp8
    WGCB = np.zeros((128, 16, 512))
    for layer in range(4):
        s, ll = divmod(layer, 2)
        wl_ = gcb_w[s, ll]
        for kc in range(4):
            WGCB[:, layer * 4 + kc, :] = wl_[kc * 128:(kc + 1) * 128]
    c["WGCB"] = q8(WGCB)

    W7At = np.zeros((128, 4, 256))
    for kc in range(4):
        W7At[:, kc, :] = W7A[kc * 128:(kc + 1) * 128]
    c["W7AT"] = q8(W7At)

    c["WG1"] = f4(gc1_w)                     # [10, 512]
    A1B1 = np.zeros((10, 512))
    A1B1[:, :256] = A1
    A1B1[:, 256:] = B1
    c["A1B1"] = f4(A1B1)

    W2T = np.zeros((128, 2, 10))
    for mc in range(2):
        W2T[:, mc, :] = W2[mc * 128:(mc + 1) * 128]
    c["W2T"] = f4(W2T)

    c["ET"] = f4(E.T)                        # [10, 10] lhsT for dct matmul

    M1S = np.zeros((50, 3, 186))
    for cc in range(3):
        M1S[:, cc, :] = M1f[cc * 50:(cc + 1) * 50]
    c["M1S"] = f4(M1S)

    M2S = np.zeros((128, 2, 3, 10))
    M2S[:, 0] = M2[:128].reshape(128, 3, 10)
    M2S[:58, 1] = M2[128:].reshape(58, 3, 10)
    c["M2S"] = f4(M2S)

    # M3S[t, c, o3, t'] = M3[(c,t), (o3,t')]
    M3S = M3.reshape(3, 10, 3, 10).transpose(1, 0, 2, 3)
    c["M3S"] = f4(M3S)

    # block-diagonal (2-sample) transposed attention, bf16
    ATT2 = np.zeros((96, 6, 96))
    atts = [att1, attb[0, 0], attb[0, 1], attb[1, 0], attb[1, 1], att7]
    for i, a in enumerate(atts):
        ATT2[0:48, i, 0:48] = a.T
        ATT2[48:96, i, 48:96] = a.T
    c["ATT2"] = np.ascontiguousarray(ATT2, ml_dtypes.bfloat16)

    BIAS = np.zeros((128, 22))
    tanh_biases = [gc1_b, gcb_b[0, 0], gcb_b[0, 1], gcb_b[1, 0], gcb_b[1, 1]]
    for li, b in enumerate(tanh_biases):
        for mc in range(4):
            BIAS[:, li * 4 + mc] = b[mc * 128:(mc + 1) * 128]
    for mc in range(2):
        BIAS[:, 20 + mc] = hb[mc * 128:(mc + 1) * 128]
    c["BIAS"] = np.ascontiguousarray(BIAS, np.float32)
    return c


CONST_SPECS = {
    "WGCB": ((128, 16, 512), FP8),
    "W7AT": ((128, 4, 256), FP8),
    "WG1": ((10, 512), BF16),
    "A1B1": ((10, 512), BF16),
    "W2T": ((128, 2, 10), BF16),
    "ET": ((10, 10), BF16),
    "M1S": ((50, 3, 186), BF16),
    "M2S": ((128, 2, 3, 10), BF16),
    "M3S": ((10, 3, 3, 10), BF16),
    "ATT2": ((96, 6, 96), BF16),
    "BIAS": ((128, 22), F32),
}


# --------------------------------------------------------------------------
# bass program
# --------------------------------------------------------------------------

def _split_matmul_waits(raw):
    """TRN2 walrus codegen allows only one sync-wait on Matmult/Ldweights.

    Move extra waits onto EventSemaphore instructions inserted just before
    (same engine, in-order execution => semantics preserved).
    """
    import json as _json
    bir = _json.loads(raw)
    for fn in bir["functions"]:
        for bb in fn["blocks"]:
            out = []
            for inst in bb["instructions"]:
                si = inst.get("sync_info")
                if (inst.get("opcode") != "EventSemaphore"
                        and si and len(si.get("on_wait") or []) > 1):
                    waits = si["on_wait"]
                    keep, extras = waits[-1], waits[:-1]
                    ip = len(out)
                    if (inst["opcode"] == "Matmult" and out
                            and out[-1].get("opcode") == "Ldweights"
                            and out[-1].get("engine") == inst["engine"]
                            and not (out[-1].get("sync_info") or {}).get(
                                "on_wait")):
                        ip = len(out) - 1
                    for j, w in enumerate(extras):
                        out.insert(ip + j, {
                            "debug": inst.get("debug", 0),
                            "engine": inst["engine"],
                            "ins": [], "outs": [],
                            "name": f"{inst['name']}_ws{j}",
                            "opcode": "EventSemaphore",
                            "sync_info": {"on_update": [], "on_wait": [w]},
                        })
                    si["on_wait"] = [keep]
                out.append(inst)
            bb["instructions"] = out
    return _json.dumps(bir).encode()


def build_nc(spc=SPC):
    """Build the per-core Bass program for `spc` samples (multiple of 16)."""
    assert spc % 16 == 0
    n_sg = spc // 16
    nc = bass.Bass()

    xh = nc.declare_dram_parameter("xseq", [spc, T_IN, F_FEAT], BF16, isOutput=False)
    ch = {
        name: nc.declare_dram_parameter(name, list(shape), dt_, isOutput=False)
        for name, (shape, dt_) in CONST_SPECS.items()
    }
    oh = nc.declare_dram_parameter("out", [spc, DCT_N, 1, F_FEAT], F32, isOutput=True)

    with TileContext(nc) as tc:
        with (
            tc.tile_pool(name="consts", bufs=1) as consts,
            tc.tile_pool(name="seq", bufs=n_sg) as p_seq,
            tc.tile_pool(name="zsb", bufs=2) as p_zsb,
            tc.tile_pool(name="ffc", bufs=2) as p_ffc,
            tc.tile_pool(name="dct", bufs=4) as p_dct,
            tc.tile_pool(name="ysb", bufs=4) as p_y,
            tc.tile_pool(name="hsb", bufs=3) as p_h,
            tc.tile_pool(name="psb", bufs=8) as p_p,
            tc.tile_pool(name="hbig", bufs=2) as p_hbig,
            tc.tile_pool(name="osb", bufs=2) as p_osb,
            tc.tile_pool(name="ps_u", bufs=2, space="PSUM") as ps_u,
            tc.tile_pool(name="ps_zt", bufs=4, space="PSUM") as ps_zt,
            tc.tile_pool(name="ps_big", bufs=2, space="PSUM") as ps_big,
        ):
            # ---- load constants ----
            W = {}
            for name, (shape, dt_) in CONST_SPECS.items():
                t = consts.tile(list(shape), dt_, tag=name)
                nc.sync.dma_start(out=t[...], in_=ch[name][...])
                W[name] = t

            # ---- all input loads up front (never behind output stores) ----
            seqs, seq40s = [], []
            for sg in range(n_sg):
                g0 = sg * 16
                seq = p_seq.tile([T_IN, 16, F_FEAT], BF16, tag="seq",
                                 name=f"seq{sg}")
                nc.sync.dma_start(
                    out=seq[...],
                    in_=xh[g0:g0 + 16].rearrange("b t f -> t b f"),
                )
                # rows 40:50 again at base partition 0 (matmul operands
                # must start at partition 0/32/64)
                seq40 = p_seq.tile([10, 16, F_FEAT], BF16, tag="seq40",
                                   name=f"seq40_{sg}")
                nc.sync.dma_start(
                    out=seq40[...],
                    in_=xh[g0:g0 + 16, 40:50].rearrange("b t f -> t b f"),
                )
                seqs.append(seq)
                seq40s.append(seq40)

            def mm(out, lhsT, rhs, start=True, stop=True, perf_mode=None):
                nc.tensor.matmul(out=out, lhsT=lhsT, rhs=rhs,
                                 start=start, stop=stop, perf_mode=perf_mode)

            # round-robin copy engines: scalar is reserved for tanh
            cp_engines = [nc.gpsimd, nc.vector, nc.gpsimd, nc.vector,
                          nc.gpsimd]
            cp_idx = [0]

            def psum_copy(dst, src):
                eng = cp_engines[cp_idx[0] % len(cp_engines)]
                cp_idx[0] += 1
                if eng is nc.scalar:
                    eng.copy(dst, src)
                else:
                    eng.tensor_copy(dst, src)

            def phase_a(sg):
                """FFC both stages + dct. Returns (ffc_all, dcts)."""
                seq, seq40 = seqs[sg], seq40s[sg]

                # ---- FFC: Z = relu(M1f^T @ Xg^T) ----
                zp = ps_big.tile([128, 2, 256], F32, tag="big", name="zp")
                for mc, msz in ((0, 128), (1, 58)):
                    for cc in range(3):
                        mm(zp[0:msz, mc, :],
                           W["M1S"][:, cc, mc * 128:mc * 128 + msz],
                           seq[:, :, cc * 16:(cc + 1) * 16],
                           start=(cc == 0), stop=(cc == 2))
                zsb = p_zsb.tile([128, 2, 256], BF16, tag="zsb")
                nc.vector.tensor_scalar_max(zsb[:, 0, :], zp[:, 0, :], 0.0)
                nc.vector.tensor_scalar_max(zsb[0:58, 1, :], zp[0:58, 1, :], 0.0)

                # ---- FFC second stage, per o3 -> ffc_all [10=t', (s,o3,g)] ----
                ffc_all = p_ffc.tile([10, 16, 3, 16], BF16, tag="ffc")
                for o3 in range(3):
                    fp = ps_big.tile([10, 256], F32, tag="big", name="fp")
                    mm(fp[...], W["M2S"][:, 0, o3, :], zsb[:, 0, :],
                       start=True, stop=False)
                    mm(fp[...], W["M2S"][0:58, 1, o3, :], zsb[0:58, 1, :],
                       start=False, stop=False)
                    for cc in range(3):
                        mm(fp[...], W["M3S"][:, cc, o3, :],
                           seq[0:10, :, cc * 16:(cc + 1) * 16],
                           start=False, stop=(cc == 2))
                    nc.vector.tensor_copy(
                        ffc_all[:, :, o3, :],
                        fp[...].rearrange("p (s g) -> p s g", g=16))

                # ---- dct per 8-group: x^T = E @ seq[40:50] -> [10, 384] ----
                dcts = []
                for g8 in range(2):
                    dp = ps_big.tile([10, 384], F32, tag="big", name="dp")
                    mm(dp[...], W["ET"][...], seq40[:, g8 * 8:g8 * 8 + 8, :])
                    dct = p_dct.tile([10, 384], BF16, tag="dct")
                    nc.vector.tensor_copy(dct[...], dp[...])
                    dcts.append(dct)
                return ffc_all, dcts

            def gcn_layer(dct, src_y, layer, out_pool, out_tag):
                """One GCN layer for one 8-sample group, wave-ordered.

                src_y: None (gc1: input is dct) or fp8 [128, 4, 384] tile.
                Returns tanh output fp8 [128, 4(kc), 384=(pair,si,node)].
                """
                zts = [ps_zt.tile([128, 4, 96], F32, tag="zt",
                                  name=f"zt{mc}")
                       for mc in range(4)]

                us = [None] * 4
                ps = [None] * 4

                def emit_f(pair):
                    u = ps_u.tile([96, 512], F32, tag="u", name="u")
                    if src_y is None:
                        mm(u[...], dct[:, pair * 96:(pair + 1) * 96],
                           W["WG1"][...])
                    else:
                        base = (layer - 1) * 4
                        for kt in range(2):
                            mm(u[...],
                               src_y[:, 2 * kt:2 * kt + 2,
                                     pair * 96:(pair + 1) * 96],
                               W["WGCB"][:, base + 2 * kt:base + 2 * kt + 2, :],
                               start=(kt == 0), stop=(kt == 1),
                               perf_mode=DR)
                    us[pair] = u

                def emit_c(pair):
                    p = p_p.tile([96, 512], BF16, tag="p", name="p")
                    psum_copy(p[...], us[pair][...])
                    ps[pair] = p

                def emit_a(pair):
                    for mc in range(4):
                        mm(zts[mc][:, pair, :],
                           ps[pair][:, mc * 128:(mc + 1) * 128],
                           W["ATT2"][:, layer, :],
                           start=True, stop=True)

                # wave order: f0 f1 a0 f2 a1 f3 a2 a3 (PE stream)
                emit_f(0)
                emit_f(1)
                emit_c(0)
                emit_a(0)
                emit_f(2)
                emit_c(1)
                emit_a(1)
                emit_f(3)
                emit_c(2)
                emit_a(2)
                emit_c(3)
                emit_a(3)

                out8 = out_pool.tile([128, 4, 384], FP8, tag=out_tag,
                                     name=out_tag)
                for mc in range(4):
                    col = layer * 4 + mc
                    nc.scalar.activation(
                        out8[:, mc, :], zts[mc][...], AF.Tanh,
                        bias=W["BIAS"][:, col:col + 1])
                return out8

            def phase_b(sg, dcts):
                """GCN gc1 + 2 stages; returns (y1s, hb2s) with the last
                residual left unfolded (gc7 consumes y1 + hb2 linearly)."""
                y8s = [gcn_layer(dcts[g8], None, 0, p_y, "y8")
                       for g8 in range(2)]
                # stage 0 (+ residual on vector)
                has = [gcn_layer(dcts[g8], y8s[g8], 1, p_h, "h8")
                       for g8 in range(2)]
                hbs = [gcn_layer(dcts[g8], has[g8], 2, p_h, "h8")
                       for g8 in range(2)]
                y1s = []
                for g8 in range(2):
                    ynew = p_y.tile([128, 4, 384], FP8, tag="y8", name="ynew")
                    nc.vector.tensor_add(ynew[...], y8s[g8][...],
                                         hbs[g8][...])
                    y1s.append(ynew)
                # stage 1 (no residual instruction; folded into gc7)
                has2 = [gcn_layer(dcts[g8], y1s[g8], 3, p_h, "h8")
                        for g8 in range(2)]
                hb2s = [gcn_layer(dcts[g8], has2[g8], 4, p_h, "h8b")
                        for g8 in range(2)]
                return y1s, hb2s

            def phase_c(sg, ffc_all, dcts, y1s, hb2s):
                """gc7 + MLP + store for one 16-sample group."""
                g0 = sg * 16
                for g8 in range(2):
                    s0 = g8 * 8
                    dct, y1, hb2 = dcts[g8], y1s[g8], hb2s[g8]
                    hp = [ps_zt.tile([128, 8, 3, 16], F32, tag="zt",
                                     name=f"hp{mc}")
                          for mc in range(2)]
                    # term2 = (x @ A1)^T  (covers all elements; start=True)
                    for mc in range(2):
                        mm(hp[mc][...],
                           W["A1B1"][:, mc * 128:(mc + 1) * 128],
                           dct[...], start=True, stop=False)

                    u7s = [None] * 4
                    p7s = [None] * 4

                    def emit_f7(pair):
                        # u7 = (y1 + hb2) @ W7A, residual folded (linear)
                        u7 = ps_u.tile([96, 256], F32, tag="u", name="u7")
                        for si, src in enumerate((y1, hb2)):
                            for kt in range(2):
                                mm(u7[...],
                                   src[:, 2 * kt:2 * kt + 2,
                                       pair * 96:(pair + 1) * 96],
                                   W["W7AT"][:, 2 * kt:2 * kt + 2, :],
                                   start=(si == 0 and kt == 0),
                                   stop=(si == 1 and kt == 1),
                                   perf_mode=DR)
                        u7s[pair] = u7

                    def emit_c7(pair):
                        p7 = p_p.tile([96, 512], BF16, tag="p", name="p7")
                        psum_copy(p7[:, 0:256], u7s[pair][...])
                        p7s[pair] = p7

                    def emit_a7(pair):
                        for mc in range(2):
                            mm(hp[mc][:, pair * 2:pair * 2 + 2, :, :],
                               p7s[pair][:, mc * 128:(mc + 1) * 128],
                               W["ATT2"][:, 5, :], start=False, stop=False)

                    emit_f7(0)
                    emit_f7(1)
                    emit_c7(0)
                    emit_a7(0)
                    emit_f7(2)
                    emit_c7(1)
                    emit_a7(1)
                    emit_f7(3)
                    emit_c7(2)
                    emit_a7(2)
                    emit_c7(3)
                    emit_a7(3)

                    # term3 = (ffc10 @ B1)^T  (rhs cols already (s, o3, g))
                    for mc in range(2):
                        mm(hp[mc][...],
                           W["A1B1"][:, 256 + mc * 128:256 + (mc + 1) * 128],
                           ffc_all[:, s0:s0 + 8, :, :],
                           start=False, stop=True)
                    # relu(h + hb) on DVE
                    hsb = p_hbig.tile([128, 2, 384], BF16, tag="hbig")
                    for mc in range(2):
                        nc.vector.tensor_scalar(
                            out=hsb[:, mc, :],
                            in0=hp[mc][...],
                            scalar1=W["BIAS"][:, 20 + mc:21 + mc],
                            scalar2=0.0,
                            op0=mybir.AluOpType.add,
                            op1=mybir.AluOpType.max)
                    # out = (h @ W2)^T -> [10, (s,f)]
                    op = ps_big.tile([10, 384], F32, tag="big", name="op")
                    for mc in range(2):
                        mm(op[...], W["W2T"][:, mc, :], hsb[:, mc, :],
                           start=(mc == 0), stop=(mc == 1))
                    osb = p_osb.tile([10, 384], F32, tag="osb")
                    nc.vector.tensor_copy(osb[...], op[...])
                    nc.sync.dma_start(
                        out=oh[g0 + s0:g0 + s0 + 8].rearrange(
                            "b t o f -> t b (o f)"),
                        in_=osb.rearrange("p (s f) -> p s f", f=F_FEAT),
                    )

            # ---- software pipeline: emit A(sg+1) between B(sg) and C(sg)
            # so C's copies/stores overlap B(sg+1) and A feeds B early ----
            ab = phase_a(0)
            for sg in range(n_sg):
                ffc_all, dcts = ab
                y1s, hb2s = phase_b(sg, dcts)
                if sg + 1 < n_sg:
                    ab = phase_a(sg + 1)
                phase_c(sg, ffc_all, dcts, y1s, hb2s)
    _orig_to_json_bytes = nc.to_json_bytes
    nc.to_json_bytes = lambda: _split_matmul_waits(_orig_to_json_bytes())
    return nc


# --------------------------------------------------------------------------
# host entry point
# --------------------------------------------------------------------------

_CACHE = {}


def kernel(**inputs):
    assert int(inputs.get("input_n", 50)) == 50
    assert int(inputs.get("output_n", 20)) == 20
    assert int(inputs.get("itera", 1)) == 1

    import ml_dtypes
    x = np.ascontiguousarray(
        np.asarray(inputs["input_seq"], np.float32).astype(ml_dtypes.bfloat16))
    assert x.shape == (B_TOT, T_IN, F_FEAT)

    consts = _build_host_consts(inputs)

    if "nc" not in _CACHE:
        _CACHE["nc"] = build_nc(SPC)
    nc = _CACHE["nc"]

    from concourse.bass_utils import run_bass_kernel_spmd

    in_maps = []
    for i in range(N_CORES):
        m = dict(consts)
        m["xseq"] = x[i * SPC:(i + 1) * SPC]
        in_maps.append(m)

    res = run_bass_kernel_spmd(nc, in_maps, list(range(N_CORES)))
    out = np.concatenate([res.results[i]["out"] for i in range(N_CORES)], axis=0)
    return out.astype(np.float32)


# revision 10
# speedup vs baseline: 1.6978x; 1.0802x over previous
"""Trainium2 Bass kernel for nn_DAFCN (motion-prediction DAFCN forward).

Structure exploited (verified vs the reference):
  * The attention branch (wq*/wk* convs, dvb) is dead code: the reference
    computes `combined[:, :, :DCT_N]` which selects only the GCN output.
  * The FFC branch (rfft -> 1x1 conv -> relu -> irfft, first 10 steps) is
    linear-relu-linear and is folded into two small matmuls (M1f, M2/M3).
  * The iDCT + MLP are folded: h = relu(gcn_out @ A1 + ffc10 @ B1 + hb),
    out = h @ W2  with  A1 = (mlp_w1[:, :30] @ idct[:, :10]).T,
    B1 = mlp_w1[:, 30:40].T, W2 = mlp_w2[:10].T, and gc7_w folded into
    W7A = gc7_w @ A1, hb = gc7_b @ A1.

Sharding: pure data parallelism — 1024 samples / 8 cores = 128 per core,
weights replicated.

Device dataflow per core:
  * GCN state y kept transposed ("layout B": feature chunks on partitions,
    (sample, node) on the free axis), 2 samples (96 rows) per matmul group.
  * feature mix:  u = y @ w   as fp8e4 DoubleRow matmuls (K=256 per mm)
  * node mix:     z^T = u^T att^T via lhsT = u rows, rhs = att^T (bf16)
  * tanh+bias on ScalarE writing fp8 state; residuals on gpsimd/DVE.
  * Emission is wave-ordered (f0 f1 a0 f2 a1 f3 a2 a3) so the PE never
    waits long on the PSUM->SBUF copies feeding the node-mix matmuls;
    copies are spread across Scalar/DVE/Pool.
  * All input DMAs are issued up front so loads never queue behind
    output stores on the SP DMA queue.
"""

import numpy as np

import concourse.bass as bass
from concourse import mybir
from concourse.tile import TileContext

F32 = mybir.dt.float32
BF16 = mybir.dt.bfloat16
FP8 = mybir.dt.float8e4
AF = mybir.ActivationFunctionType
DR = mybir.MatmulPerfMode.DoubleRow

N_CORES = 8
B_TOT, T_IN, F_FEAT = 1024, 50, 48
SPC = B_TOT // N_CORES          # samples per core
D = 512
DCT_N = 10


# --------------------------------------------------------------------------
# host-side constant folding
# --------------------------------------------------------------------------

def _build_host_consts(inp):
    f8 = np.float64
    w1 = np.asarray(inp["mlp_w1"], f8)       # [256, 40]
    w2 = np.asarray(inp["mlp_w2"], f8)       # [40, 256]
    wg = np.asarray(inp["ffc_wg"], f8)       # [6, 6]
    wl = np.asarray(inp["ffc_wl"], f8)       # [3, 3]
    gc1_w = np.asarray(inp["gc1_w"], f8)     # [10, 512]
    gc1_b = np.asarray(inp["gc1_b"], f8)     # [512]
    gcb_w = np.asarray(inp["gcb_w"], f8)     # [2, 2, 512, 512]
    gcb_b = np.asarray(inp["gcb_b"], f8)     # [2, 2, 512]
    gc7_w = np.asarray(inp["gc7_w"], f8)     # [512, 10]
    gc7_b = np.asarray(inp["gc7_b"], f8)     # [10]
    att1 = np.asarray(inp["gc1_att"], f8)    # [48, 48]
    attb = np.asarray(inp["gcb_att"], f8)    # [2, 2, 48, 48]
    att7 = np.asarray(inp["gc7_att"], f8)    # [48, 48]

    # DCT pair (block length 30)
    N = 30
    kk = np.arange(N)[:, None]
    ii = np.arange(N)[None, :]
    w = np.full((N, 1), np.sqrt(2.0 / N))
    w[0, 0] = np.sqrt(1.0 / N)
    d = w * np.cos(np.pi * (ii + 0.5) * kk / N)
    idct = np.linalg.inv(d)
    dct10 = d[:DCT_N]                        # [10, 30]

    # E: x^T[d, f] = sum_j E[d, j] * seq[40+j, f]
    E = dct10[:, :10].copy()
    E[:, 9] += dct10[:, 10:].sum(axis=1)

    # MLP folds
    A1 = (w1[:, :30] @ idct[:, :10]).T       # [10, 256]
    B1 = w1[:, 30:40].T                      # [10, 256]
    W7A = gc7_w @ A1                         # [512, 256]
    hb = gc7_b @ A1                          # [256]
    W2 = w2[:10].T                           # [256, 10]

    # FFC fold: rfft / channel mix / (relu) / irfft+local, first 10 steps
    Fm = np.fft.rfft(np.eye(60), axis=-1)    # [60, 31]
    Fr, Fi = Fm.real.T, Fm.imag.T            # [31, 60]
    M1 = (np.einsum("oc,kt->ctok", wg[:, :3], Fr)
          + np.einsum("oc,kt->ctok", wg[:, 3:], Fi)).reshape(3, 60, 186)
    M1f = np.concatenate(
        [M1[:, :49], M1[:, 49:].sum(axis=1, keepdims=True)], axis=1
    ).reshape(150, 186)                      # [(c,t<50), (o,k)]
    Gr = np.fft.irfft(np.eye(31), n=60, axis=-1)[:, :10]
    Gi = np.fft.irfft(1j * np.eye(31), n=60, axis=-1)[:, :10]
    M2 = np.zeros((6, 31, 3, 10))
    for o3 in range(3):
        M2[o3, :, o3, :] = Gr
        M2[o3 + 3, :, o3, :] = Gi
    M2 = M2.reshape(186, 30)
    M3 = np.einsum("oc,tu->ctou", wl, np.eye(10)).reshape(30, 30)

    import ml_dtypes
    c = {}
    f4 = lambda a: np.ascontiguousarray(a, ml_dtypes.bfloat16)
    q8 = lambda a: np.ascontiguousarray(a, ml_dtypes.float8_e4m3)

    # WGCB [128, 16, 512]: (layer l, k-chunk kc) -> w_l[kc*128+p, j], fp8
    WGCB = np.zeros((128, 16, 512))
    for layer in range(4):
        s, ll = divmod(layer, 2)
        wl_ = gcb_w[s, ll]
        for kc in range(4):
            WGCB[:, layer * 4 + kc, :] = wl_[kc * 128:(kc + 1) * 128]
    c["WGCB"] = q8(WGCB)

    W7At = np.zeros((128, 4, 256))
    for kc in range(4):
        W7At[:, kc, :] = W7A[kc * 128:(kc + 1) * 128]
    c["W7AT"] = q8(W7At)

    c["WG1"] = f4(gc1_w)                     # [10, 512]
    A1B1 = np.zeros((10, 512))
    A1B1[:, :256] = A1
    A1B1[:, 256:] = B1
    c["A1B1"] = f4(A1B1)

    W2T = np.zeros((128, 2, 10))
    for mc in range(2):
        W2T[:, mc, :] = W2[mc * 128:(mc + 1) * 128]
    c["W2T"] = f4(W2T)

    c["ET"] = f4(E.T)                        # [10, 10] lhsT for dct matmul

    M1S = np.zeros((50, 3, 186))
    for cc in range(3):
        M1S[:, cc, :] = M1f[cc * 50:(cc + 1) * 50]
    c["M1S"] = f4(M1S)

    M2S = np.zeros((128, 2, 3, 10))
    M2S[:, 0] = M2[:128].reshape(128, 3, 10)
    M2S[:58, 1] = M2[128:].reshape(58, 3, 10)
    c["M2S"] = f4(M2S)

    # M3S[t, c, o3, t'] = M3[(c,t), (o3,t')]
    M3S = M3.reshape(3, 10, 3, 10).transpose(1, 0, 2, 3)
    c["M3S"] = f4(M3S)

    # block-diagonal (2-sample) transposed attention, bf16
    ATT2 = np.zeros((96, 6, 96))
    atts = [att1, attb[0, 0], attb[0, 1], attb[1, 0], attb[1, 1], att7]
    for i, a in enumerate(atts):
        ATT2[0:48, i, 0:48] = a.T
        ATT2[48:96, i, 48:96] = a.T
    c["ATT2"] = np.ascontiguousarray(ATT2, ml_dtypes.bfloat16)

    BIAS = np.zeros((128, 22))
    tanh_biases = [gc1_b, gcb_b[0, 0], gcb_b[0, 1], gcb_b[1, 0], gcb_b[1, 1]]
    for li, b in enumerate(tanh_biases):
        for mc in range(4):
            BIAS[:, li * 4 + mc] = b[mc * 128:(mc + 1) * 128]
    for mc in range(2):
        BIAS[:, 20 + mc] = hb[mc * 128:(mc + 1) * 128]
    c["BIAS"] = np.ascontiguousarray(BIAS, np.float32)
    return c


CONST_SPECS = {
    "WGCB": ((128, 16, 512), FP8),
    "W7AT": ((128, 4, 256), FP8),
    "WG1": ((10, 512), BF16),
    "A1B1": ((10, 512), BF16),
    "W2T": ((128, 2, 10), BF16),
    "ET": ((10, 10), BF16),
    "M1S": ((50, 3, 186), BF16),
    "M2S": ((128, 2, 3, 10), BF16),
    "M3S": ((10, 3, 3, 10), BF16),
    "ATT2": ((96, 6, 96), BF16),
    "BIAS": ((128, 22), F32),
}


# --------------------------------------------------------------------------
# bass program
# --------------------------------------------------------------------------

def _split_matmul_waits(raw):
    """TRN2 walrus codegen allows only one sync-wait on Matmult/Ldweights.

    Move extra waits onto EventSemaphore instructions inserted just before
    (same engine, in-order execution => semantics preserved).
    """
    import json as _json
    bir = _json.loads(raw)
    for fn in bir["functions"]:
        for bb in fn["blocks"]:
            out = []
            for inst in bb["instructions"]:
                si = inst.get("sync_info")
                if (inst.get("opcode") != "EventSemaphore"
                        and si and len(si.get("on_wait") or []) > 1):
                    waits = si["on_wait"]
                    keep, extras = waits[-1], waits[:-1]
                    ip = len(out)
                    if (inst["opcode"] == "Matmult" and out
                            and out[-1].get("opcode") == "Ldweights"
                            and out[-1].get("engine") == inst["engine"]
                            and not (out[-1].get("sync_info") or {}).get(
                                "on_wait")):
                        ip = len(out) - 1
                    for j, w in enumerate(extras):
                        out.insert(ip + j, {
                            "debug": inst.get("debug", 0),
                            "engine": inst["engine"],
                            "ins": [], "outs": [],
                            "name": f"{inst['name']}_ws{j}",
                            "opcode": "EventSemaphore",
                            "sync_info": {"on_update": [], "on_wait": [w]},
                        })
                    si["on_wait"] = [keep]
                out.append(inst)
            bb["instructions"] = out
    return _json.dumps(bir).encode()


def build_nc(spc=SPC):
    """Build the per-core Bass program for `spc` samples (multiple of 16)."""
    assert spc % 16 == 0
    n_sg = spc // 16
    nc = bass.Bass()

    xh = nc.declare_dram_parameter("xseq", [spc, T_IN, F_FEAT], BF16, isOutput=False)
    ch = {
        name: nc.declare_dram_parameter(name, list(shape), dt_, isOutput=False)
        for name, (shape, dt_) in CONST_SPECS.items()
    }
    oh = nc.declare_dram_parameter("out", [spc, DCT_N, 1, F_FEAT], F32, isOutput=True)

    with TileContext(nc) as tc:
        with (
            tc.tile_pool(name="consts", bufs=1) as consts,
            tc.tile_pool(name="seq", bufs=n_sg) as p_seq,
            tc.tile_pool(name="zsb", bufs=2) as p_zsb,
            tc.tile_pool(name="ffc", bufs=2) as p_ffc,
            tc.tile_pool(name="dct", bufs=4) as p_dct,
            tc.tile_pool(name="ysb", bufs=4) as p_y,
            tc.tile_pool(name="hsb", bufs=3) as p_h,
            tc.tile_pool(name="psb", bufs=8) as p_p,
            tc.tile_pool(name="hbig", bufs=2) as p_hbig,
            tc.tile_pool(name="osb", bufs=2) as p_osb,
            tc.tile_pool(name="ps_u", bufs=3, space="PSUM") as ps_u,
            tc.tile_pool(name="ps_zt", bufs=4, space="PSUM") as ps_zt,
            tc.tile_pool(name="ps_big", bufs=1, space="PSUM") as ps_big,
        ):
            # ---- load constants ----
            W = {}
            for name, (shape, dt_) in CONST_SPECS.items():
                t = consts.tile(list(shape), dt_, tag=name)
                nc.sync.dma_start(out=t[...], in_=ch[name][...])
                W[name] = t

            # ---- all input loads up front (never behind output stores) ----
            seqs, seq40s = [], []
            for sg in range(n_sg):
                g0 = sg * 16
                seq = p_seq.tile([T_IN, 16, F_FEAT], BF16, tag="seq",
                                 name=f"seq{sg}")
                nc.sync.dma_start(
                    out=seq[...],
                    in_=xh[g0:g0 + 16].rearrange("b t f -> t b f"),
                )
                # rows 40:50 again at base partition 0 (matmul operands
                # must start at partition 0/32/64)
                seq40 = p_seq.tile([10, 16, F_FEAT], BF16, tag="seq40",
                                   name=f"seq40_{sg}")
                nc.sync.dma_start(
                    out=seq40[...],
                    in_=xh[g0:g0 + 16, 40:50].rearrange("b t f -> t b f"),
                )
                seqs.append(seq)
                seq40s.append(seq40)

            def mm(out, lhsT, rhs, start=True, stop=True, perf_mode=None):
                nc.tensor.matmul(out=out, lhsT=lhsT, rhs=rhs,
                                 start=start, stop=stop, perf_mode=perf_mode)

            # round-robin copy engines: scalar is reserved for tanh
            cp_engines = [nc.gpsimd, nc.vector, nc.gpsimd, nc.vector,
                          nc.gpsimd]
            cp_idx = [0]

            def psum_copy(dst, src):
                eng = cp_engines[cp_idx[0] % len(cp_engines)]
                cp_idx[0] += 1
                if eng is nc.scalar:
                    eng.copy(dst, src)
                else:
                    eng.tensor_copy(dst, src)

            def phase_a(sg):
                """FFC both stages + dct. Returns (ffc_all, dcts)."""
                seq, seq40 = seqs[sg], seq40s[sg]

                # ---- FFC: Z = relu(M1f^T @ Xg^T) ----
                zp = ps_big.tile([128, 2, 256], F32, tag="big", name="zp")
                for mc, msz in ((0, 128), (1, 58)):
                    for cc in range(3):
                        mm(zp[0:msz, mc, :],
                           W["M1S"][:, cc, mc * 128:mc * 128 + msz],
                           seq[:, :, cc * 16:(cc + 1) * 16],
                           start=(cc == 0), stop=(cc == 2))
                zsb = p_zsb.tile([128, 2, 256], BF16, tag="zsb")
                nc.vector.tensor_scalar_max(zsb[:, 0, :], zp[:, 0, :], 0.0)
                nc.vector.tensor_scalar_max(zsb[0:58, 1, :], zp[0:58, 1, :], 0.0)

                # ---- FFC second stage, per o3 -> ffc_all [10=t', (s,o3,g)] ----
                ffc_all = p_ffc.tile([10, 16, 3, 16], BF16, tag="ffc")
                for o3 in range(3):
                    fp = ps_big.tile([10, 256], F32, tag="big", name="fp")
                    mm(fp[...], W["M2S"][:, 0, o3, :], zsb[:, 0, :],
                       start=True, stop=False)
                    mm(fp[...], W["M2S"][0:58, 1, o3, :], zsb[0:58, 1, :],
                       start=False, stop=False)
                    for cc in range(3):
                        mm(fp[...], W["M3S"][:, cc, o3, :],
                           seq[0:10, :, cc * 16:(cc + 1) * 16],
                           start=False, stop=(cc == 2))
                    nc.vector.tensor_copy(
                        ffc_all[:, :, o3, :],
                        fp[...].rearrange("p (s g) -> p s g", g=16))

                # ---- dct per 8-group: x^T = E @ seq[40:50] -> [10, 384] ----
                dcts = []
                for g8 in range(2):
                    dp = ps_big.tile([10, 384], F32, tag="big", name="dp")
                    mm(dp[...], W["ET"][...], seq40[:, g8 * 8:g8 * 8 + 8, :])
                    dct = p_dct.tile([10, 384], BF16, tag="dct")
                    nc.vector.tensor_copy(dct[...], dp[...])
                    dcts.append(dct)
                return ffc_all, dcts

            def gcn_layer(dct, src_y, layer, out_pool, out_tag):
                """One GCN layer for one 8-sample group, wave-ordered.

                src_y: None (gc1: input is dct) or fp8 [128, 4, 384] tile.
                Returns tanh output fp8 [128, 4(kc), 384=(pair,si,node)].
                """
                zts = [ps_zt.tile([128, 4, 96], F32, tag="zt",
                                  name=f"zt{mc}")
                       for mc in range(4)]

                us = [None] * 4
                ps = [None] * 4

                def emit_f(pair):
                    u = ps_u.tile([96, 512], F32, tag="u", name="u")
                    if src_y is None:
                        mm(u[...], dct[:, pair * 96:(pair + 1) * 96],
                           W["WG1"][...])
                    else:
                        base = (layer - 1) * 4
                        for kt in range(2):
                            mm(u[...],
                               src_y[:, 2 * kt:2 * kt + 2,
                                     pair * 96:(pair + 1) * 96],
                               W["WGCB"][:, base + 2 * kt:base + 2 * kt + 2, :],
                               start=(kt == 0), stop=(kt == 1),
                               perf_mode=DR)
                    us[pair] = u

                def emit_c(pair):
                    p = p_p.tile([96, 512], BF16, tag="p", name="p")
                    psum_copy(p[...], us[pair][...])
                    ps[pair] = p

                def emit_a(pair):
                    for mc in range(4):
                        mm(zts[mc][:, pair, :],
                           ps[pair][:, mc * 128:(mc + 1) * 128],
                           W["ATT2"][:, layer, :],
                           start=True, stop=True)

                # wave order: f0 f1 f2 a0 f3 a1 a2 a3 (PE stream)
                emit_f(0)
                emit_f(1)
                emit_c(0)
                emit_f(2)
                emit_c(1)
                emit_a(0)
                emit_f(3)
                emit_c(2)
                emit_a(1)
                emit_c(3)
                emit_a(2)
                emit_a(3)

                out8 = out_pool.tile([128, 4, 384], FP8, tag=out_tag,
                                     name=out_tag)
                for mc in range(4):
                    col = layer * 4 + mc
                    nc.scalar.activation(
                        out8[:, mc, :], zts[mc][...], AF.Tanh,
                        bias=W["BIAS"][:, col:col + 1])
                return out8

            def phase_b(sg, dcts):
                """GCN gc1 + 2 stages; returns (y1s, hb2s) with the last
                residual left unfolded (gc7 consumes y1 + hb2 linearly)."""
                y8s = [gcn_layer(dcts[g8], None, 0, p_y, "y8")
                       for g8 in range(2)]
                # stage 0 (+ residual on vector)
                has = [gcn_layer(dcts[g8], y8s[g8], 1, p_h, "h8")
                       for g8 in range(2)]
                hbs = [gcn_layer(dcts[g8], has[g8], 2, p_h, "h8")
                       for g8 in range(2)]
                y1s = []
                for g8 in range(2):
                    ynew = p_y.tile([128, 4, 384], FP8, tag="y8", name="ynew")
                    nc.vector.tensor_add(ynew[...], y8s[g8][...],
                                         hbs[g8][...])
                    y1s.append(ynew)
                # stage 1 (no residual instruction; folded into gc7)
                has2 = [gcn_layer(dcts[g8], y1s[g8], 3, p_h, "h8")
                        for g8 in range(2)]
                hb2s = [gcn_layer(dcts[g8], has2[g8], 4, p_h, "h8b")
                        for g8 in range(2)]
                return y1s, hb2s

            def phase_c(sg, ffc_all, dcts, y1s, hb2s):
                """gc7 + MLP + store for one 16-sample group."""
                g0 = sg * 16
                for g8 in range(2):
                    s0 = g8 * 8
                    dct, y1, hb2 = dcts[g8], y1s[g8], hb2s[g8]
                    hp = [ps_zt.tile([128, 8, 3, 16], F32, tag="zt",
                                     name=f"hp{mc}")
                          for mc in range(2)]
                    # term2 = (x @ A1)^T  (covers all elements; start=True)
                    for mc in range(2):
                        mm(hp[mc][...],
                           W["A1B1"][:, mc * 128:(mc + 1) * 128],
                           dct[...], start=True, stop=False)

                    u7s = [None] * 4
                    p7s = [None] * 4

                    def emit_f7(pair):
                        # u7 = (y1 + hb2) @ W7A, residual folded (linear)
                        u7 = ps_u.tile([96, 256], F32, tag="u", name="u7")
                        for si, src in enumerate((y1, hb2)):
                            for kt in range(2):
                                mm(u7[...],
                                   src[:, 2 * kt:2 * kt + 2,
                                       pair * 96:(pair + 1) * 96],
                                   W["W7AT"][:, 2 * kt:2 * kt + 2, :],
                                   start=(si == 0 and kt == 0),
                                   stop=(si == 1 and kt == 1),
                                   perf_mode=DR)
                        u7s[pair] = u7

                    def emit_c7(pair):
                        p7 = p_p.tile([96, 512], BF16, tag="p", name="p7")
                        psum_copy(p7[:, 0:256], u7s[pair][...])
                        p7s[pair] = p7

                    def emit_a7(pair):
                        for mc in range(2):
                            mm(hp[mc][:, pair * 2:pair * 2 + 2, :, :],
                               p7s[pair][:, mc * 128:(mc + 1) * 128],
                               W["ATT2"][:, 5, :], start=False, stop=False)

                    emit_f7(0)
                    emit_f7(1)
                    emit_c7(0)
                    emit_f7(2)
                    emit_c7(1)
                    emit_a7(0)
                    emit_f7(3)
                    emit_c7(2)
                    emit_a7(1)
                    emit_c7(3)
                    emit_a7(2)
                    emit_a7(3)

                    # term3 = (ffc10 @ B1)^T  (rhs cols already (s, o3, g))
                    for mc in range(2):
                        mm(hp[mc][...],
                           W["A1B1"][:, 256 + mc * 128:256 + (mc + 1) * 128],
                           ffc_all[:, s0:s0 + 8, :, :],
                           start=False, stop=True)
                    # relu(h + hb) on DVE
                    hsb = p_hbig.tile([128, 2, 384], BF16, tag="hbig")
                    for mc in range(2):
                        nc.vector.tensor_scalar(
                            out=hsb[:, mc, :],
                            in0=hp[mc][...],
                            scalar1=W["BIAS"][:, 20 + mc:21 + mc],
                            scalar2=0.0,
                            op0=mybir.AluOpType.add,
                            op1=mybir.AluOpType.max)
                    # out = (h @ W2)^T -> [10, (s,f)]
                    op = ps_big.tile([10, 384], F32, tag="big", name="op")
                    for mc in range(2):
                        mm(op[...], W["W2T"][:, mc, :], hsb[:, mc, :],
                           start=(mc == 0), stop=(mc == 1))
                    osb = p_osb.tile([10, 384], F32, tag="osb")
                    nc.vector.tensor_copy(osb[...], op[...])
                    nc.sync.dma_start(
                        out=oh[g0 + s0:g0 + s0 + 8].rearrange(
                            "b t o f -> t b (o f)"),
                        in_=osb.rearrange("p (s f) -> p s f", f=F_FEAT),
                    )

            # ---- software pipeline: emit A(sg+1) between B(sg) and C(sg)
            # so C's copies/stores overlap B(sg+1) and A feeds B early ----
            ab = phase_a(0)
            for sg in range(n_sg):
                ffc_all, dcts = ab
                y1s, hb2s = phase_b(sg, dcts)
                if sg + 1 < n_sg:
                    ab = phase_a(sg + 1)
                phase_c(sg, ffc_all, dcts, y1s, hb2s)
    _orig_to_json_bytes = nc.to_json_bytes
    nc.to_json_bytes = lambda: _split_matmul_waits(_orig_to_json_bytes())
    return nc


# --------------------------------------------------------------------------
# host entry point
# --------------------------------------------------------------------------

_CACHE = {}


def kernel(**inputs):
    assert int(inputs.get("input_n", 50)) == 50
    assert int(inputs.get("output_n", 20)) == 20
    assert int(inputs.get("itera", 1)) == 1

    import ml_dtypes
    x = np.ascontiguousarray(
        np.asarray(inputs["input_seq"], np.float32).astype(ml_dtypes.bfloat16))
    assert x.shape == (B_TOT, T_IN, F_FEAT)

    consts = _build_host_consts(inputs)

    if "nc" not in _CACHE:
        _CACHE["nc"] = build_nc(SPC)
    nc = _CACHE["nc"]

    from concourse.bass_utils import run_bass_kernel_spmd

    in_maps = []
    for i in range(N_CORES):
        m = dict(consts)
        m["xseq"] = x[i * SPC:(i + 1) * SPC]
        in_maps.append(m)

    res = run_bass_kernel_spmd(nc, in_maps, list(range(N_CORES)))
    out = np.concatenate([res.results[i]["out"] for i in range(N_CORES)], axis=0)
    return out.astype(np.float32)
